# revision 1
# baseline (speedup 1.0000x reference)
"""LRU (Linear Recurrent Unit) forward pass on 8 Trainium2 NeuronCores.

Reference computation (shapes: x [4096, 4, 1024]):
  u        = einsum("nbd,ed->nbe", x, Wi) + bi          # (n, b, 2d)
  u_r, u_i = gamma * u[..., 0::2], gamma * u[..., 1::2]  # complex input, per channel
  h[t]     = lam * h[t-1] + u[t]                         # complex diagonal recurrence
  out      = einsum("nbe,de->nbd", [h_r, h_i], Wo) + bo

Strategy:
  - Shard 4-way over batch x 2-way over the channel dim (8 cores). Each core
    computes a partial output for its 512 channels; host sums the two halves.
  - lam = nu * exp(i*theta) is factored into a real magnitude scan (native
    tensor_tensor_scan: state = nu*state + w, per-partition lane) and
    unit-modulus rotations e^{+-i*theta*t} applied elementwise with
    host-precomputed cos/sin tables (block-local time, tables reused across
    blocks; scan state is rotated by e^{i*theta*T} between blocks).
  - Projections run on the PE in float32r (1 cycle/row); weights are
    pre-transposed/reordered on the host, with gamma and the even/odd
    deinterleave folded into Wi and the bias applied during PSUM eviction
    on the scalar engine.
"""

import sys

sys.path.insert(0, "/opt/trn_rl_repo")

import numpy as np

import concourse.bass as bass
import concourse.mybir as mybir
from concourse.tile import TileContext
from concourse.vector_clock import ScopedClock

N_SEQ = 4096
BATCH = 4
D = 1024
C = D // 2          # channels per core (d-split by 2)
T = 256             # tokens per block
NBLK = N_SEQ // T
NCT = C // 128      # channel tiles per core (4)
NET = 2 * C // 128  # e-tiles per core (8): 4 real + 4 imag
NDT = D // 128      # d tiles (8)
NMT = D // 128      # output m tiles (8)

F32 = mybir.dt.float32
F32R = mybir.dt.float32r
AF = mybir.ActivationFunctionType
ALU = mybir.AluOpType

# ---------------------------------------------------------------------------
# Walrus in this container accepts at most ONE sync wait per instruction.
# Patch the Tile tail drain and post-process the BIR to split excess waits
# onto same-engine nops.
# ---------------------------------------------------------------------------
_WAIT_LIMIT = 1
_uid = [0]


def _patched_drain_and_barrier(self, tick_clock, wait_clock):
    probe = self.nc.sync.nop(nofuse=True)
    wait_clock.add_sem_waits(probe.ins, ScopedClock({None: tick_clock.global_clock}))
    si = probe.ins.sync_info
    waits = list(si.on_wait) if si is not None else []
    if si is not None:
        probe.ins.sync_info = mybir.SyncInfo(
            on_wait=waits[:_WAIT_LIMIT], on_update=list(si.on_update)
        )
    for c in range(_WAIT_LIMIT, len(waits), _WAIT_LIMIT):
        nop = self.nc.sync.nop(nofuse=True)
        nop.ins.sync_info = mybir.SyncInfo(
            on_wait=waits[c : c + _WAIT_LIMIT], on_update=[]
        )
    self.nc.sync.drain()
    self.nc.all_engine_barrier()
    assert self.sems is not None
    popped = self.nc._tile_sem_poison_stack.pop()
    assert popped is self._sem_poison
    self.nc.clear_and_free_semaphores(list(self.sems.allocated().values()))
    self.nc.all_engine_barrier()


TileContext._drain_and_barrier = _patched_drain_and_barrier


def _split_sync_waits(nc):
    for func in nc.m.functions:
        for bb in func.blocks:
            out = []
            changed = False
            for ins in bb.instructions:
                si = ins.sync_info
                if si is not None and len(si.on_wait) > _WAIT_LIMIT:
                    waits = list(si.on_wait)
                    for w in waits[:-_WAIT_LIMIT]:
                        _uid[0] += 1
                        nop = mybir.InstNoOp(name=f"wsplit-{_uid[0]}")
                        nop.engine = ins.engine
                        nop.sync_info = mybir.SyncInfo(on_wait=[w], on_update=[])
                        out.append(nop)
                        changed = True
                    ins.sync_info = mybir.SyncInfo(
                        on_wait=waits[-_WAIT_LIMIT:], on_update=list(si.on_update)
                    )
                out.append(ins)
            if changed:
                bb.instructions = out


# ---------------------------------------------------------------------------
# Bass program (identical on all 8 cores; per-core data differs)
# ---------------------------------------------------------------------------
def _build_program():
    nc = bass.Bass()

    xT = nc.dram_tensor("xT", [D, N_SEQ], F32R, kind="ExternalInput")
    wiT = nc.dram_tensor("wiT", [D, 2 * C], F32R, kind="ExternalInput")
    woT = nc.dram_tensor("woT", [2 * C, D], F32R, kind="ExternalInput")
    ubias = nc.dram_tensor("ubias", [128, NET], F32, kind="ExternalInput")
    obias = nc.dram_tensor("obias", [128, NMT], F32, kind="ExternalInput")
    cosT = nc.dram_tensor("cosT", [C, T], F32, kind="ExternalInput")
    sinT = nc.dram_tensor("sinT", [C, T], F32, kind="ExternalInput")
    nuT = nc.dram_tensor("nuT", [C, T], F32, kind="ExternalInput")
    ctst = nc.dram_tensor("ctst", [128, 2 * NCT], F32, kind="ExternalInput")
    outT = nc.dram_tensor("outT", [D, N_SEQ], F32, kind="ExternalOutput")

    with TileContext(nc) as tc:
        with (
            tc.tile_pool(name="const", bufs=1) as cpool,
            tc.tile_pool(name="xt", bufs=3) as xt_pool,
            tc.tile_pool(name="u", bufs=2) as u_pool,
            tc.tile_pool(name="w", bufs=2) as w_pool,
            tc.tile_pool(name="s", bufs=2) as s_pool,
            tc.tile_pool(name="h", bufs=2) as h_pool,
            tc.tile_pool(name="o", bufs=2) as o_pool,
            tc.tile_pool(name="tmp", bufs=4) as tmp_pool,
            tc.tile_pool(name="z", bufs=1) as z_pool,
            tc.tile_pool(name="pu", bufs=3, space="PSUM") as pu_pool,
            tc.tile_pool(name="po", bufs=3, space="PSUM") as po_pool,
        ):
            # resident constants
            wi_sb = []
            for dk in range(NDT):
                t = cpool.tile([128, 2 * C], F32R, tag=f"wi{dk}")
                nc.sync.dma_start(out=t[:], in_=wiT[dk * 128 : (dk + 1) * 128, :])
                wi_sb.append(t)
            wo_sb = []
            for et in range(NET):
                t = cpool.tile([128, D], F32R, tag=f"wo{et}")
                nc.sync.dma_start(out=t[:], in_=woT[et * 128 : (et + 1) * 128, :])
                wo_sb.append(t)
            cos_sb, sin_sb, nu_sb = [], [], []
            for ct in range(NCT):
                rows = slice(ct * 128, (ct + 1) * 128)
                tcb = cpool.tile([128, T], F32, tag=f"cos{ct}")
                nc.sync.dma_start(out=tcb[:], in_=cosT[rows, :])
                cos_sb.append(tcb)
                tsb = cpool.tile([128, T], F32, tag=f"sin{ct}")
                nc.sync.dma_start(out=tsb[:], in_=sinT[rows, :])
                sin_sb.append(tsb)
                tnb = cpool.tile([128, T], F32, tag=f"nu{ct}")
                nc.sync.dma_start(out=tnb[:], in_=nuT[rows, :])
                nu_sb.append(tnb)
            ub_sb = cpool.tile([128, NET], F32, tag="ubias")
            nc.sync.dma_start(out=ub_sb[:], in_=ubias[:])
            ob_sb = cpool.tile([128, NMT], F32, tag="obias")
            nc.sync.dma_start(out=ob_sb[:], in_=obias[:])
            zrot_sb = cpool.tile([128, 2 * NCT], F32, tag="ctst")
            nc.sync.dma_start(out=zrot_sb[:], in_=ctst[:])

            # carry state (scan state at the end of the previous block,
            # rotated by e^{i*theta*T}); one column per channel tile
            zr = z_pool.tile([128, NCT], F32, tag="zr")
            zi = z_pool.tile([128, NCT], F32, tag="zi")

            for blk in range(NBLK):
                t0 = blk * T
                # ---- load x^T tiles for this block
                xt = []
                for dk in range(NDT):
                    t = xt_pool.tile([128, T], F32R, tag=f"xt{dk}")
                    nc.sync.dma_start(
                        out=t[:], in_=xT[dk * 128 : (dk + 1) * 128, t0 : t0 + T]
                    )
                    xt.append(t)
                # ---- input projection: u^T[e, t] on PE, bias added on evict
                u = []
                for et in range(NET):
                    pt = pu_pool.tile([128, T], F32, tag="pu")
                    for dk in range(NDT):
                        nc.tensor.matmul(
                            pt[:],
                            wi_sb[dk][:, et * 128 : (et + 1) * 128],
                            xt[dk][:],
                            start=(dk == 0),
                            stop=(dk == NDT - 1),
                        )
                    ut = u_pool.tile([128, T], F32, tag=f"u{et}")
                    nc.scalar.activation(
                        ut[:], pt[:], AF.Identity, bias=ub_sb[:, et : et + 1]
                    )
                    u.append(ut)
                # ---- pre-rotation (gpsimd): w = e^{-i theta t} * u
                w = []
                for ct in range(NCT):
                    ur, ui = u[ct], u[NCT + ct]
                    ta = tmp_pool.tile([128, T], F32, tag="ta")
                    tb = tmp_pool.tile([128, T], F32, tag="tb")
                    wr = w_pool.tile([128, T], F32, tag=f"wr{ct}")
                    wi = w_pool.tile([128, T], F32, tag=f"wi{ct}")
                    nc.gpsimd.tensor_mul(ta[:], cos_sb[ct][:], ur[:])
                    nc.gpsimd.tensor_mul(tb[:], sin_sb[ct][:], ui[:])
                    nc.gpsimd.tensor_add(wr[:], ta[:], tb[:])
                    nc.gpsimd.tensor_mul(ta[:], cos_sb[ct][:], ui[:])
                    nc.gpsimd.tensor_mul(tb[:], sin_sb[ct][:], ur[:])
                    nc.gpsimd.tensor_sub(wi[:], ta[:], tb[:])
                    w.append((wr, wi))
                # ---- magnitude scan (DVE), carry chained across blocks
                s = []
                for ct in range(NCT):
                    wr, wi = w[ct]
                    sr = s_pool.tile([128, T], F32, tag=f"sr{ct}")
                    si = s_pool.tile([128, T], F32, tag=f"si{ct}")
                    init_r = 0.0 if blk == 0 else zr[:, ct : ct + 1]
                    init_i = 0.0 if blk == 0 else zi[:, ct : ct + 1]
                    nc.vector.tensor_tensor_scan(
                        sr[:], nu_sb[ct][:], wr[:], init_r, ALU.mult, ALU.add
                    )
                    nc.vector.tensor_tensor_scan(
                        si[:], nu_sb[ct][:], wi[:], init_i, ALU.mult, ALU.add
                    )
                    s.append((sr, si))
                # ---- carry for next block: z = e^{i theta T} * s[:, T-1]
                if blk < NBLK - 1:
                    for ct in range(NCT):
                        sr, si = s[ct]
                        cT_ap = zrot_sb[:, ct : ct + 1]
                        sT_ap = zrot_sb[:, NCT + ct : NCT + ct + 1]
                        tz = tmp_pool.tile([128, 1], F32, tag="tz")
                        nc.vector.tensor_scalar_mul(tz[:], si[:, T - 1 : T], sT_ap)
                        nc.vector.scalar_tensor_tensor(
                            zr[:, ct : ct + 1],
                            sr[:, T - 1 : T],
                            cT_ap,
                            tz[:],
                            ALU.mult,
                            ALU.subtract,
                        )
                        tz2 = tmp_pool.tile([128, 1], F32, tag="tz")
                        nc.vector.tensor_scalar_mul(tz2[:], sr[:, T - 1 : T], sT_ap)
                        nc.vector.scalar_tensor_tensor(
                            zi[:, ct : ct + 1],
                            si[:, T - 1 : T],
                            cT_ap,
                            tz2[:],
                            ALU.mult,
                            ALU.add,
                        )
                # ---- post-rotation (DVE): h = e^{+i theta t} * s
                h = [None] * NET
                for ct in range(NCT):
                    sr, si = s[ct]
                    ta = tmp_pool.tile([128, T], F32, tag="ta")
                    tb = tmp_pool.tile([128, T], F32, tag="tb")
                    hr = h_pool.tile([128, T], F32, tag=f"hr{ct}")
                    hi = h_pool.tile([128, T], F32, tag=f"hi{ct}")
                    nc.vector.tensor_mul(ta[:], cos_sb[ct][:], sr[:])
                    nc.vector.tensor_mul(tb[:], sin_sb[ct][:], si[:])
                    nc.vector.tensor_sub(hr[:].bitcast(F32R), ta[:], tb[:])
                    nc.vector.tensor_mul(ta[:], cos_sb[ct][:], si[:])
                    nc.vector.tensor_mul(tb[:], sin_sb[ct][:], sr[:])
                    nc.vector.tensor_add(hi[:].bitcast(F32R), ta[:], tb[:])
                    h[ct] = hr
                    h[NCT + ct] = hi
                # ---- output projection: out^T[m, t] += Wo^T . h
                for mt in range(NMT):
                    pt = po_pool.tile([128, T], F32, tag="po")
                    for et in range(NET):
                        nc.tensor.matmul(
                            pt[:],
                            wo_sb[et][:, mt * 128 : (mt + 1) * 128],
                            h[et][:].bitcast(F32R),
                            start=(et == 0),
                            stop=(et == NET - 1),
                        )
                    ot = o_pool.tile([128, T], F32, tag=f"o{mt}")
                    nc.scalar.activation(
                        ot[:], pt[:], AF.Identity, bias=ob_sb[:, mt : mt + 1]
                    )
                    nc.sync.dma_start(
                        out=outT[mt * 128 : (mt + 1) * 128, t0 : t0 + T], in_=ot[:]
                    )

    _split_sync_waits(nc)
    return nc


_CACHED = None


def _get_program():
    global _CACHED
    if _CACHED is None:
        _CACHED = _build_program()
    return _CACHED


# ---------------------------------------------------------------------------
# Host-side sharding / weight preparation
# ---------------------------------------------------------------------------
def make_in_maps(x, nu_log, theta_log, gamma_log, Wi, bi, Wo, bo):
    nu = np.exp(nu_log.astype(np.float64))          # decay magnitude per channel
    theta = np.exp(theta_log.astype(np.float64))    # rotation angle per channel
    gamma = np.exp(-np.exp(gamma_log.astype(np.float64)))

    tt = np.arange(T, dtype=np.float64)
    in_maps = []
    for core in range(8):
        bi_idx = core % 4
        half = core // 4
        cs = np.arange(half * C, (half + 1) * C)    # global channels

        xTc = np.ascontiguousarray(x[:, bi_idx, :].T)  # [D, N_SEQ]

        g = gamma[cs].astype(np.float32)
        wiTc = np.empty((D, 2 * C), dtype=np.float32)
        wiTc[:, :C] = (Wi[2 * cs, :] * g[:, None]).T
        wiTc[:, C:] = (Wi[2 * cs + 1, :] * g[:, None]).T
        ub = np.concatenate([g * bi[2 * cs], g * bi[2 * cs + 1]]).astype(np.float32)
        ub2 = np.ascontiguousarray(ub.reshape(NET, 128).T)

        woTc = np.empty((2 * C, D), dtype=np.float32)
        woTc[:C, :] = Wo[:, cs].T
        woTc[C:, :] = Wo[:, D + cs].T
        ob = bo.astype(np.float32) if half == 0 else np.zeros(D, np.float32)
        ob2 = np.ascontiguousarray(ob.reshape(NMT, 128).T)

        ang = theta[cs][:, None] * tt[None, :]       # [C, T]
        cosTc = np.cos(ang).astype(np.float32)
        sinTc = np.sin(ang).astype(np.float32)
        nuTc = np.broadcast_to(
            nu[cs].astype(np.float32)[:, None], (C, T)
        ).copy()
        angT = theta[cs] * T
        ctstc = np.empty((128, 2 * NCT), dtype=np.float32)
        for ct in range(NCT):
            rows = slice(ct * 128, (ct + 1) * 128)
            ctstc[:, ct] = np.cos(angT[rows])
            ctstc[:, NCT + ct] = np.sin(angT[rows])

        in_maps.append(
            {
                "xT": xTc,
                "wiT": wiTc,
                "woT": woTc,
                "ubias": ub2,
                "obias": ob2,
                "cosT": cosTc,
                "sinT": sinTc,
                "nuT": nuTc,
                "ctst": ctstc,
            }
        )
    return in_maps


def assemble_output(results):
    out = np.empty((N_SEQ, BATCH, D), dtype=np.float32)
    for bi_idx in range(BATCH):
        acc = results[bi_idx]["outT"] + results[4 + bi_idx]["outT"]  # [D, N_SEQ]
        out[:, bi_idx, :] = acc.T
    return out


def kernel(x, nu_log, theta_log, gamma_log, Wi, bi, Wo, bo):
    from concourse.bass_utils import run_bass_kernel_spmd

    nc = _get_program()
    in_maps = make_in_maps(x, nu_log, theta_log, gamma_log, Wi, bi, Wo, bo)
    res = run_bass_kernel_spmd(nc, in_maps, list(range(8)))
    return assemble_output(res.results)
